# revision 35
# baseline (speedup 1.0000x reference)
"""Trainium2 Bass kernel v3 for nn_CBFRICL (multi-behavior GCN + CL/BPR losses).

v3 over v2:
 * 4 SWDGE queues (Bacc(num_swdge_queues=4)): dma_gather descriptor
   generation runs on all 4 Q7 core pairs concurrently (~4x gather wall).
 * Gather calls issued per (tile-group, window) so each dst tile's PSUM
   accumulation chain spans all 3 windows: no per-window DVE adds, one
   PSUM->SBUF copy per tile, done on the Scalar (ACT) engine.
 * One-hot matrices built per CHUNK (not per slot); slot matmuls use
   partition-sliced operands [a:b) to select their tile's token range.
 * bf16 table (rows padded to 256B for the gather), bf16 messages/onehots/
   aggregation matmuls/W-apply. PSUM accumulation stays fp32.
 * dinv scaling via Scalar engine activation(Copy, scale=dinv_col).
 * Next-behavior T AllGather triggered before the loss-phase te AllGather.

Node tables keep the p-major interleaved DRAM layout: local node
l = t*128 + p of a core's slice maps to DRAM row p*88 + t of that core's
block; gather/loss indices are precomputed on the host accordingly.
"""

import sys
import types
import numpy as np

import concourse.bass as bass
import concourse.bacc as bacc
import concourse.mybir as mybir
import concourse.tile as tile
from concourse import library_config  # noqa: F401
from concourse.bass_utils import run_bass_kernel_spmd
from concourse.tile import TileContext

# ---------------------------------------------------------------- constants
N_CORES = 8
D = 64
N_USERS = 50000
N_ITEMS = 40000
N_NODES = (N_USERS + 1) + (N_ITEMS + 1)      # 90002
NTAB = 90112                                  # padded to 704*128
SLICE = NTAB // N_CORES                       # 11264 rows per core
TILES = SLICE // 128                          # 88 tiles per slice
NB = 3
L = 2
B = 8192
BS = B // N_CORES                             # 1024 batch rows per core
WIN = 32768                                   # int16 gather window
NWIN = 3
GT = 8                                        # tiles per gather group
NG = TILES // GT                              # 11 groups
NQ = 4                                        # SWDGE queues
OH_GROUP = 16                                 # slots per DVE onehot op
SENT = 200.0                                  # sentinel dest col (no match)
REG_WEIGHT = 0.01
CL_TEMP = 0.1
DT = mybir.dt.float32
BF = mybir.dt.bfloat16
I16 = mybir.dt.int16
I32 = mybir.dt.int32
DP = 128                                      # padded bf16 row elems (256B)

_PATCHED = False


def _install_walrus_patch():
    """The walrus in this toolchain accepts at most ONE sync-wait per
    instruction; Tile attaches several. Spread extras across injected
    same-engine nops after TileContext exits."""
    global _PATCHED
    if _PATCHED:
        return
    _PATCHED = True

    def split_waits(nc):
        for bb in nc.main_func.blocks:
            out = []
            for inst in list(bb.instructions):
                si = inst.sync_info
                if (
                    si is not None
                    and si.on_wait
                    and len(si.on_wait) > 1
                    and inst.engine in nc.engines
                ):
                    extra = list(si.on_wait[1:])
                    si.on_wait = list(si.on_wait[:1])
                    for w in extra:
                        nop = nc.engines[inst.engine].nop(nofuse=True)
                        nopi = nop.ins
                        for b2 in nc.main_func.blocks:
                            if nopi in b2.instructions:
                                b2.instructions.remove(nopi)
                                break
                        nopi.sync_info = mybir.SyncInfo(on_wait=[w], on_update=[])
                        out.append(nopi)
                out.append(inst)
            bb.instructions[:] = out

    orig_exit = TileContext.__exit__

    def patched_exit(self, *a, **k):
        r = orig_exit(self, *a, **k)
        split_waits(self.nc)
        return r

    TileContext.__exit__ = patched_exit


LAST_EXEC_NS = None
LAST_DIAG = None


def _try_install_trace_shim():
    try:
        import antenv.axon_hooks  # noqa: F401
        return True
    except ImportError:
        pass
    try:
        import antenv
        mod = types.ModuleType("antenv.axon_hooks")
        _hook = [None]
        mod.set_axon_ntff_profile_hook = lambda h: _hook.__setitem__(0, h)
        mod.get_axon_ntff_profile_hook = lambda: _hook[0]
        sys.modules["antenv.axon_hooks"] = mod
        antenv.axon_hooks = mod
        from trn_agent_boot.trn_boot import _ntff_profile_via_ctypes
        mod.set_axon_ntff_profile_hook(
            _ntff_profile_via_ctypes("/opt/axon/libaxon_pjrt.so"))
        return True
    except Exception:
        return False


# ------------------------------------------------------------- host helpers
def _wrap_idx(a):
    """[n] -> [128, n//16] wrapped int16 layout (token j -> partition j%16,
    col j//16; replicated across the 8 Q7 core groups)."""
    a = np.ascontiguousarray(a, np.int16)
    n = len(a)
    return np.tile(a.reshape(n // 16, 16).T, (8, 1))


def _irow(node):
    """Global node id -> interleaved DRAM row (p-major within core block)."""
    c = node // SLICE
    l = node - c * SLICE
    return c * SLICE + (l % 128) * TILES + l // 128


def _build_plan(edge_index):
    """Per-(group, window) gather schedule + per-core payloads.

    plan[b]:
      calls: [{g, w, ntok, nch, tok_base, chunk_base, seg{tile:pos}}]
      qa:    queue per call (greedy token balance)
      tslots: {tile: [{call, chunk, a, b, col, start, stop}]}
              (emission: per tile, call order; col = global onehot column)
      nslot, tottok
    core_data[core]: gidx {b: int16 [128, tottok/16]},
                     dcol {b: f32 [128, nslot]}
    """
    plan = []
    core_data = [dict(gidx={}, dcol={}) for _ in range(N_CORES)]
    for b in range(NB):
        rows = np.asarray(edge_index[b, 0], np.int64)
        cols = np.asarray(edge_index[b, 1], np.int64)
        r = _irow(rows)
        w = r // WIN
        ridx = r - w * WIN
        core_of = cols // SLICE
        lcol = cols - core_of * SLICE
        t = lcol // 128
        p = lcol % 128
        key = (core_of * TILES + t) * NWIN + w
        order = np.argsort(key, kind="stable")
        ridx_s, p_s = ridx[order], p[order]
        counts = np.bincount(
            key[order], minlength=N_CORES * TILES * NWIN
        ).reshape(N_CORES, TILES, NWIN)
        offs = np.zeros(N_CORES * TILES * NWIN + 1, np.int64)
        np.cumsum(counts.reshape(-1), out=offs[1:])
        L_tw = counts.max(axis=0)               # [TILES, NWIN]

        MAXCH = 16                              # chunks per gather piece
        calls = []
        spans = []
        tslots = {ti: [] for ti in range(TILES)}
        tok_base = 0
        for g in range(NG):
            tl = list(range(g * GT, (g + 1) * GT))
            raw = {ti: [] for ti in tl}         # slots per tile, call order
            for wi in range(NWIN):
                span = int(L_tw[tl[0]:tl[-1] + 1, wi].sum())
                if span == 0:
                    continue
                ntok = ((span + 127) // 128) * 128
                nch = ntok // 128
                first_piece = len(calls)
                for p0 in range(0, nch, MAXCH):
                    pch = min(MAXCH, nch - p0)
                    calls.append(dict(g=g, w=wi, ntok=pch * 128, nch=pch,
                                      tok_base=tok_base + p0 * 128))
                seg = {}
                pos = 0
                for ti in tl:
                    n = int(L_tw[ti, wi])
                    seg[ti] = pos
                    if n:
                        c0, c1 = pos // 128, (pos + n - 1) // 128
                        for c in range(c0, c1 + 1):
                            a = max(pos, c * 128) - c * 128
                            bb = min(pos + n, (c + 1) * 128) - c * 128
                            raw[ti].append(dict(
                                call=first_piece + c // MAXCH,
                                chunk=c % MAXCH, a=a, b=bb))
                    pos += n
                spans.append(dict(w=wi, tok_base=tok_base, seg=seg))
                tok_base += ntok
            for ti in tl:
                sl = raw[ti]
                for k, s in enumerate(sl):
                    s["start"] = k == 0
                    s["stop"] = k == len(sl) - 1
                tslots[ti] = sl
        tottok = tok_base
        # global onehot column per slot, in device emission order
        nslot = 0
        for g in range(NG):
            for ti in range(g * GT, (g + 1) * GT):
                for s in tslots[ti]:
                    s["col"] = nslot
                    nslot += 1

        # greedy queue balance by tokens
        qload = [0] * NQ
        qa = []
        for c in calls:
            q = min(range(NQ), key=lambda i: qload[i])
            qa.append(q)
            qload[q] += c["ntok"]

        plan.append(dict(calls=calls, qa=qa, tslots=tslots,
                         nslot=nslot, tottok=tottok))

        # ---- per-core payloads
        for core in range(N_CORES):
            gidx = np.zeros(tottok, np.int64)
            # token -> dst partition (255 = not this core's token)
            pc = np.full(tottok, 255, np.int64)
            for sp in spans:
                wi = sp["w"]
                tb = sp["tok_base"]
                for ti, s0 in sp["seg"].items():
                    n = int(counts[core, ti, wi])
                    if n == 0:
                        continue
                    o = offs[(core * TILES + ti) * NWIN + wi]
                    gidx[tb + s0:tb + s0 + n] = ridx_s[o:o + n]
                    pc[tb + s0:tb + s0 + n] = p_s[o:o + n]
            dcol = np.full((nslot, 128), SENT, np.float32)
            for ti in range(TILES):
                for s in tslots[ti]:
                    tb = calls[s["call"]]["tok_base"]
                    c0 = tb + s["chunk"] * 128
                    seg = pc[c0 + s["a"]:c0 + s["b"]]
                    dcol[s["col"], s["a"]:s["b"]] = np.where(
                        seg == 255, SENT, seg)
            core_data[core]["gidx"][b] = _wrap_idx(gidx)
            core_data[core]["dcol"][b] = np.ascontiguousarray(dcol.T)
    return plan, core_data


# ------------------------------------------------------------ device kernel
def _build_program(plan):
    nc = bacc.Bacc(num_swdge_queues=NQ)

    maxc = max(c["nch"] for pb in plan for c in pb["calls"])

    x0 = nc.declare_dram_parameter("x0", [128, TILES * D], DT, isOutput=False)
    dinv_in = nc.declare_dram_parameter("dinv", [NB, 128, TILES], DT, isOutput=False)
    wts = nc.declare_dram_parameter("wts", [NB, L, D, D], DT, isOutput=False)
    bias = nc.declare_dram_parameter("bias", [NB, L, 128, D], DT, isOutput=False)
    umask = nc.declare_dram_parameter("umask", [128, TILES], DT, isOutput=False)
    imask = nc.declare_dram_parameter("imask", [128, TILES], DT, isOutput=False)
    iota_in = nc.declare_dram_parameter("iota", [128, 128], DT, isOutput=False)
    g_par = {}
    d_par = {}
    for b in range(NB):
        g_par[b] = nc.declare_dram_parameter(
            f"gidx_{b}", [128, plan[b]["tottok"] // 16], I16, isOutput=False)
        d_par[b] = nc.declare_dram_parameter(
            f"dcol_{b}", [128, plan[b]["nslot"]], DT, isOutput=False)
    bidx = nc.declare_dram_parameter(
        "bidx", [NB, 3, BS // 128, 128, 1], I32, isOutput=False)
    y = nc.declare_dram_parameter("y", [1, 8], DT, isOutput=True)

    with tile.TileContext(nc) as tc:
        with (
            tc.tile_pool(name="sb", bufs=3) as sb,
            tc.tile_pool(name="cst", bufs=1) as cst,
            tc.tile_pool(name="beh", bufs=1) as beh,
            tc.tile_pool(name="ohp", bufs=6) as ohp,
            tc.tile_pool(name="msgp", bufs=12) as msgp,
            tc.tile_pool(name="lossp", bufs=1) as lossp,
            tc.tile_pool(name="idxp", bufs=4) as idxp,
            tc.tile_pool(name="psA", bufs=3, space="PSUM") as psA,
            tc.tile_pool(name="psB", bufs=2, space="PSUM") as psB,
            tc.tile_pool(name="psC", bufs=1, space="PSUM") as psC,
            tc.tile_pool(name="dram", bufs=1, space="DRAM") as dram,
        ):
            # ---- persistent SBUF state
            te_sl = cst.tile([128, TILES * D], DT, tag="te")
            ttab = cst.tile([128, TILES * D], BF, tag="ttab")
            sfm = cst.tile([64, TILES * 128], BF, tag="sfm")

            # ---- DRAM work buffers
            ag_in = [dram.tile([SLICE, DP], BF, name=f"ag_in{i}")
                     for i in range(2)]
            te_ag = dram.tile([SLICE, D], DT)
            T_full = {}
            te_full = {}
            for _b in range(NB):
                for _l in range(L):
                    T_full[_b, _l] = dram.tile(
                        [NTAB, DP], BF, addr_space="Shared",
                        name=f"T_full_{_b}_{_l}")
                te_full[_b] = dram.tile(
                    [NTAB, D], DT, addr_space="Shared", name=f"te_full_{_b}")
            oh_store = {
                _b: dram.tile([128, plan[_b]["nslot"] * 128], BF,
                              name=f"oh_store_{_b}")
                for _b in range(NB)}

            # ---- constants
            iota_f = sb.tile([128, 128], DT, tag="iotaf")
            nc.sync.dma_start(out=iota_f[:], in_=iota_in[:, :])
            iota_sb = cst.tile([128, OH_GROUP * 128], BF, tag="iota")
            for _k in range(OH_GROUP):
                nc.vector.tensor_copy(
                    iota_sb[:, _k * 128:(_k + 1) * 128], iota_f[:])
            umask_sb = cst.tile([128, TILES], DT, tag="umask")
            nc.sync.dma_start(out=umask_sb[:], in_=umask[:, :])
            imask_sb = cst.tile([128, TILES], DT, tag="imask")
            nc.sync.dma_start(out=imask_sb[:], in_=imask[:, :])

            # acc: 0=cl+bpr, 1=ssu, 2=ssi (rest unused)
            acc = cst.tile([128, 8], DT, tag="acc")
            nc.vector.memset(acc[:], 0.0)
            eps24 = cst.tile([128, 1], DT, tag="eps24")
            nc.vector.memset(eps24[:], 1e-24)
            eps16 = cst.tile([128, 1], DT, tag="eps16")
            nc.vector.memset(eps16[:], 1e-16)

            # ---- init: te = x0 (one DMA); emb-loss partials
            nc.sync.dma_start(out=te_sl[:], in_=x0[:, :])
            rs_all = cst.tile([128, TILES], DT, tag="rsall")
            H = TILES // 2
            for hh in range(2):
                sqa = lossp.tile([128, H * D], DT, tag="sqa")
                nc.vector.tensor_mul(
                    sqa[:], te_sl[:, hh * H * D:(hh + 1) * H * D],
                    te_sl[:, hh * H * D:(hh + 1) * H * D])
                nc.vector.reduce_sum(
                    rs_all[:, hh * H:(hh + 1) * H],
                    sqa[:].rearrange("p (a d) -> p a d", d=D),
                    axis=mybir.AxisListType.X)
            mtmp = sb.tile([128, TILES], DT, tag="mtmp")
            nc.vector.tensor_mul(mtmp[:], rs_all[:], umask_sb[:])
            nc.vector.reduce_sum(acc[:, 1:2], mtmp[:], axis=mybir.AxisListType.X)
            nc.vector.tensor_mul(mtmp[:], rs_all[:], imask_sb[:])
            nc.vector.reduce_sum(acc[:, 2:3], mtmp[:], axis=mybir.AxisListType.X)

            agi = [0]  # ag_in rotation counter

            def emit_loss(b):
                """Loss lookups + CL/BPR math for behavior b (needs te_full[b])."""
                uf = lossp.tile([128, (BS // 128) * D], DT, tag="uf")
                pf = lossp.tile([128, (BS // 128) * D], DT, tag="pf")
                nf = lossp.tile([128, (BS // 128) * D], DT, tag="nf")
                for which, dstt in ((0, uf), (1, pf), (2, nf)):
                    for t in range(BS // 128):
                        ix = idxp.tile([128, 1], I32, tag="bix")
                        nc.sync.dma_start(out=ix[:], in_=bidx[b, which, t])
                        nc.gpsimd.indirect_dma_start(
                            out=dstt[:, t * D:(t + 1) * D], out_offset=None,
                            in_=te_full[b][:, :],
                            in_offset=bass.IndirectOffsetOnAxis(
                                ap=ix[:, :1], axis=0))

                dots = {}
                for nm, a_, b_ in (("dup", uf, pf), ("dun", uf, nf),
                                   ("nu", uf, uf), ("npos", pf, pf),
                                   ("nneg", nf, nf)):
                    dt_ = lossp.tile([128, BS // 128], DT, tag="dt" + nm)
                    m = lossp.tile([128, (BS // 128) * D], DT, tag="dotm")
                    nc.vector.tensor_mul(m[:], a_[:], b_[:])
                    nc.vector.reduce_sum(
                        dt_[:], m[:].rearrange("p (a d) -> p a d", d=D),
                        axis=mybir.AxisListType.X)
                    dots[nm] = dt_

                NC8 = BS // 128
                den = sb.tile([128, NC8], DT, tag="den")
                nc.vector.tensor_mul(den[:], dots["nu"][:], dots["npos"][:])
                den2 = sb.tile([128, NC8], DT, tag="den2")
                nc.vector.tensor_mul(den2[:], dots["nu"][:], dots["nneg"][:])
                nc.scalar.activation(
                    den[:], den[:], mybir.ActivationFunctionType.Sqrt,
                    bias=eps16[:, 0:1])
                nc.scalar.activation(
                    den2[:], den2[:], mybir.ActivationFunctionType.Sqrt,
                    bias=eps16[:, 0:1])
                nc.vector.reciprocal(den[:], den[:])
                nc.vector.reciprocal(den2[:], den2[:])
                cosp = sb.tile([128, NC8], DT, tag="cosp")
                nc.vector.tensor_mul(cosp[:], dots["dup"][:], den[:])
                cosn = sb.tile([128, NC8], DT, tag="cosn")
                nc.vector.tensor_mul(cosn[:], dots["dun"][:], den2[:])
                # cl = log(1 + exp((cosn - cosp)/T))
                dlt = sb.tile([128, NC8], DT, tag="dlt")
                nc.vector.tensor_sub(dlt[:], cosn[:], cosp[:])
                nc.vector.tensor_scalar_mul(dlt[:], dlt[:], 1.0 / CL_TEMP)
                # bpr = min(softplus(dun-dup), -ln(1e-10))
                dsc = sb.tile([128, NC8], DT, tag="dsc")
                nc.vector.tensor_sub(dsc[:], dots["dun"][:], dots["dup"][:])
                nc.scalar.activation(
                    dlt[:], dlt[:], mybir.ActivationFunctionType.Exp)
                nc.scalar.activation(
                    dsc[:], dsc[:], mybir.ActivationFunctionType.Exp)
                nc.vector.tensor_scalar_add(dlt[:], dlt[:], 1.0)
                nc.vector.tensor_scalar_add(dsc[:], dsc[:], 1.0)
                nc.scalar.activation(
                    dlt[:], dlt[:], mybir.ActivationFunctionType.Ln)
                nc.scalar.activation(
                    dsc[:], dsc[:], mybir.ActivationFunctionType.Ln)
                nc.vector.tensor_scalar_min(dsc[:], dsc[:], 23.02585)
                nc.vector.tensor_add(dlt[:], dlt[:], dsc[:])
                lsum = sb.tile([128, 1], DT, tag="lsum")
                nc.vector.reduce_sum(lsum[:], dlt[:], axis=mybir.AxisListType.X)
                nc.vector.tensor_add(acc[:, 0:1], acc[:, 0:1], lsum[:])

            def ag_view(buf):
                return buf[:].rearrange("(p a) d -> p a d", p=128)

            def publish_ttab(buf, g):
                t0, t1 = g * GT, (g + 1) * GT
                nc.sync.dma_start(
                    out=ag_view(buf)[:, t0:t1, 0:D],
                    in_=ttab[:, t0 * D:t1 * D].rearrange(
                        "p (a d) -> p a d", d=D))

            for b in range(NB):
                pb = plan[b]
                dinv_sb = beh.tile([128, TILES], DT, tag="dinv")
                nc.sync.dma_start(out=dinv_sb[:], in_=dinv_in[b])
                gidx_sb = beh.tile([128, pb["tottok"] // 16], I16, tag="gidx")
                nc.sync.dma_start(out=gidx_sb[:], in_=g_par[b][:, :])
                dcol_f = beh.tile([128, pb["nslot"]], DT, tag="dcolf")
                nc.sync.dma_start(out=dcol_f[:], in_=d_par[b][:, :])
                dcol_sb = beh.tile([128, pb["nslot"]], BF, tag="dcol")
                nc.vector.tensor_copy(dcol_sb[:], dcol_f[:])

                if b == 0:
                    # T(0,0) = dinv * te; publish all groups + AG
                    buf = ag_in[agi[0] % 2]
                    agi[0] += 1
                    for t in range(TILES):
                        nc.scalar.activation(
                            ttab[:, t * D:(t + 1) * D],
                            te_sl[:, t * D:(t + 1) * D],
                            mybir.ActivationFunctionType.Copy,
                            scale=dinv_sb[:, t:t + 1])
                    for g in range(NG):
                        publish_ttab(buf, g)
                    nc.gpsimd.collective_compute(
                        "AllGather", mybir.AluOpType.bypass,
                        replica_groups=[list(range(N_CORES))],
                        ins=[buf[:]], outs=[T_full[0, 0][:]])

                for l in range(L):
                    last = l == L - 1
                    w_sb = sb.tile([D, D], BF, tag="wsb")
                    nc.gpsimd.dma_start(out=w_sb[:], in_=wts[b, l])
                    bi_sb = sb.tile([128, D], DT, tag="bisb")
                    nc.sync.dma_start(out=bi_sb[:], in_=bias[b, l])
                    if last and b + 1 < NB:
                        dinv_nb = beh.tile([128, TILES], DT, tag="dinvn")
                        nc.sync.dma_start(out=dinv_nb[:], in_=dinv_in[b + 1])
                    nxt_buf = None
                    if not (last and b + 1 == NB):
                        nxt_buf = ag_in[agi[0] % 2]
                        agi[0] += 1

                    pending = [None]

                    def flush_post():
                        if pending[0] is not None:
                            pending[0]()
                            pending[0] = None

                    for g in range(NG):
                        gcalls = [ci for ci, c in enumerate(pb["calls"])
                                  if c["g"] == g]
                        # ---- gathers for this group
                        msgs = {}
                        for ci in gcalls:
                            call = pb["calls"][ci]
                            wlen = min(WIN, NTAB - call["w"] * WIN)
                            src = T_full[b, l][
                                call["w"] * WIN:call["w"] * WIN + wlen, :]
                            gi = gidx_sb[:, call["tok_base"] // 16:
                                         (call["tok_base"] + call["ntok"]) // 16]
                            msg = msgp.tile([128, maxc * DP], BF, tag="msg")
                            nc.gpsimd.dma_gather(
                                out_ap=msg[:, 0:call["nch"] * DP].rearrange(
                                    "p (c d) -> p c d", d=DP),
                                in_ap=src, idxs_ap=gi,
                                num_idxs=call["ntok"],
                                num_idxs_reg=call["ntok"],
                                elem_size=DP, single_packet=False,
                                queue_num=pb["qa"][ci])
                            msgs[ci] = msg

                        # ---- onehot builds per slot (batched)
                        gslots = [s for ti in range(g * GT, (g + 1) * GT)
                                  for s in pb["tslots"][ti]]
                        s0g = gslots[0]["col"]
                        nsg = len(gslots)
                        ohtiles = []
                        for j0 in range(0, nsg, OH_GROUP):
                            k = min(OH_GROUP, nsg - j0)
                            oh = ohp.tile([128, OH_GROUP * 128], BF, tag="oh")
                            if l == 0:
                                nc.vector.tensor_tensor(
                                    out=oh[:, 0:k * 128].rearrange(
                                        "p (k c) -> p k c", c=128),
                                    in0=iota_sb[:, 0:k * 128].rearrange(
                                        "p (k c) -> p k c", c=128),
                                    in1=dcol_sb[:, s0g + j0:s0g + j0 + k
                                                ].unsqueeze(2).broadcast_to(
                                                    (128, k, 128)),
                                    op=mybir.AluOpType.is_equal)
                                nc.sync.dma_start(
                                    out=oh_store[b][
                                        :, (s0g + j0) * 128:
                                        (s0g + j0 + k) * 128],
                                    in_=oh[:, 0:k * 128])
                            else:
                                nc.sync.dma_start(
                                    out=oh[:, 0:k * 128],
                                    in_=oh_store[b][
                                        :, (s0g + j0) * 128:
                                        (s0g + j0 + k) * 128])
                            ohtiles.append(oh)

                        # previous group's DVE post, now that its inputs are
                        # long done — keeps DVE from head-of-line blocking
                        flush_post()

                        # ---- per-tile chains + W-apply into group batch
                        t0, t1 = g * GT, (g + 1) * GT
                        x2g = sb.tile([128, GT * D], DT, tag="x2g")
                        for ti in range(t0, t1):
                            sl = pb["tslots"][ti]
                            if sl:
                                agg = psA.tile([64, 128], DT, tag="agg")
                                for s in sl:
                                    msg = msgs[s["call"]]
                                    gc = s["col"] - s0g
                                    oh = ohtiles[gc // OH_GROUP]
                                    jj = gc % OH_GROUP
                                    nc.tensor.matmul(
                                        out=agg[:],
                                        lhsT=msg[:, s["chunk"] * DP:
                                                 s["chunk"] * DP + D],
                                        rhs=oh[:, jj * 128:(jj + 1) * 128],
                                        start=s["start"], stop=s["stop"])
                                nc.scalar.activation(
                                    sfm[:, ti * 128:(ti + 1) * 128], agg[:],
                                    mybir.ActivationFunctionType.Copy)
                            else:
                                nc.vector.memset(
                                    sfm[:, ti * 128:(ti + 1) * 128], 0.0)
                            z = psB.tile([128, D], DT, tag="zps")
                            nc.tensor.matmul(
                                out=z[:], lhsT=sfm[:, ti * 128:(ti + 1) * 128],
                                rhs=w_sb[:], start=True, stop=True)
                            li = ti - t0
                            nc.scalar.activation(
                                x2g[:, li * D:(li + 1) * D], z[:],
                                mybir.ActivationFunctionType.Copy,
                                scale=dinv_sb[:, ti:ti + 1])

                        # ---- batched group post (deferred one group)
                        def post(g=g, x2g=x2g, t0=t0, t1=t1):
                            x3 = x2g[:].rearrange("p (a d) -> p a d", d=D)
                            nc.vector.tensor_tensor(
                                out=x3, in0=x3,
                                in1=bi_sb[:].unsqueeze(1).broadcast_to(
                                    (128, GT, D)),
                                op=mybir.AluOpType.add)
                            if not last:
                                nc.vector.tensor_tensor(
                                    out=ttab[:, t0 * D:t1 * D].rearrange(
                                        "p (a d) -> p a d", d=D),
                                    in0=x3,
                                    in1=dinv_sb[:, t0:t1].unsqueeze(
                                        2).broadcast_to((128, GT, D)),
                                    op=mybir.AluOpType.mult)
                            else:
                                sq = sb.tile([128, GT * D], DT, tag="sq")
                                nc.vector.tensor_mul(sq[:], x2g[:], x2g[:])
                                n2 = sb.tile([128, GT], DT, tag="n2")
                                nc.vector.reduce_sum(
                                    n2[:], sq[:].rearrange(
                                        "p (a d) -> p a d", d=D),
                                    axis=mybir.AxisListType.X)
                                nc.scalar.activation(
                                    n2[:], n2[:],
                                    mybir.ActivationFunctionType.Sqrt,
                                    bias=eps24[:, 0:1])
                                nc.vector.reciprocal(n2[:], n2[:])
                                le = sb.tile([128, GT * D], DT, tag="le")
                                nc.vector.tensor_tensor(
                                    out=le[:].rearrange("p (a d) -> p a d", d=D),
                                    in0=x3,
                                    in1=n2[:].unsqueeze(2).broadcast_to(
                                        (128, GT, D)),
                                    op=mybir.AluOpType.mult)
                                nc.vector.tensor_add(
                                    te_sl[:, t0 * D:t1 * D],
                                    te_sl[:, t0 * D:t1 * D], le[:])
                                if b + 1 < NB:
                                    nc.vector.tensor_tensor(
                                        out=ttab[:, t0 * D:t1 * D].rearrange(
                                            "p (a d) -> p a d", d=D),
                                        in0=te_sl[:, t0 * D:t1 * D].rearrange(
                                            "p (a d) -> p a d", d=D),
                                        in1=dinv_nb[:, t0:t1].unsqueeze(
                                            2).broadcast_to((128, GT, D)),
                                        op=mybir.AluOpType.mult)
                            if not (last and b + 1 == NB):
                                publish_ttab(nxt_buf, g)

                        pending[0] = post

                        # deferred te exchange + losses of the previous
                        # behavior, hidden under this layer's gathers. The
                        # dep-DMA reading T_full[b,0] serializes the te AG
                        # behind the critical T AG on the collective engine.
                        if l == 0 and b > 0 and g == 3:
                            nc.gpsimd.dma_start(
                                out=te_ag[0:1, 0:1],
                                in_=T_full[b, 0][0:1, 0:1])
                            nc.sync.dma_start(
                                out=te_ag[:].rearrange("(p a) d -> p (a d)",
                                                       p=128),
                                in_=te_sl[:])
                            nc.gpsimd.collective_compute(
                                "AllGather", mybir.AluOpType.bypass,
                                replica_groups=[list(range(N_CORES))],
                                ins=[te_ag[:]], outs=[te_full[b - 1][:]])
                            emit_loss(b - 1)

                    flush_post()

                    # ---- layer-end collectives (T is the critical path)
                    if not last:
                        nc.gpsimd.collective_compute(
                            "AllGather", mybir.AluOpType.bypass,
                            replica_groups=[list(range(N_CORES))],
                            ins=[nxt_buf[:]], outs=[T_full[b, l + 1][:]])
                    elif b + 1 < NB:
                        nc.gpsimd.collective_compute(
                            "AllGather", mybir.AluOpType.bypass,
                            replica_groups=[list(range(N_CORES))],
                            ins=[nxt_buf[:]], outs=[T_full[b + 1, 0][:]])
                    else:
                        nc.sync.dma_start(
                            out=te_ag[:].rearrange("(p a) d -> p (a d)", p=128),
                            in_=te_sl[:])
                        nc.gpsimd.collective_compute(
                            "AllGather", mybir.AluOpType.bypass,
                            replica_groups=[list(range(N_CORES))],
                            ins=[te_ag[:]], outs=[te_full[b][:]])
                        emit_loss(b)

            # ---- final reduction: partition-sum acc via matmul with ones
            ones = cst.tile([128, 1], DT, tag="ones")
            nc.vector.memset(ones[:], 1.0)
            red_ps = psC.tile([1, 8], DT, tag="redps")
            nc.tensor.matmul(
                out=red_ps[:], lhsT=ones[:], rhs=acc[:], start=True, stop=True)
            red = sb.tile([1, 8], DT, tag="red")
            nc.vector.tensor_copy(red[:], red_ps[:])
            out_b = dram.tile([1, 8], DT)
            red_all = dram.tile([1, 8], DT, addr_space="Shared")
            nc.gpsimd.dma_start(out=out_b[:], in_=red[:])
            nc.gpsimd.collective_compute(
                "AllReduce", mybir.AluOpType.add,
                replica_groups=[list(range(N_CORES))],
                ins=[out_b[:]], outs=[red_all[:]])
            fin = sb.tile([1, 8], DT, tag="fin")
            nc.sync.dma_start(out=fin[:], in_=red_all[:])
            nc.sync.dma_start(out=y[:], in_=fin[:])
    return nc


# ------------------------------------------------------------------ kernel()
def kernel(user_emb, item_emb, gcn_w, gcn_b, edge_index, batch_data):
    _install_walrus_patch()

    user_emb = np.asarray(user_emb, np.float32)
    item_emb = np.asarray(item_emb, np.float32)
    gcn_w = np.asarray(gcn_w, np.float32)
    gcn_b = np.asarray(gcn_b, np.float32)
    edge_index = np.asarray(edge_index)
    batch_data = np.asarray(batch_data)

    x0 = np.zeros((NTAB, D), np.float32)
    x0[:N_USERS + 1] = user_emb
    x0[N_USERS + 1:N_NODES] = item_emb

    plan, core_data = _build_plan(edge_index)

    deg = np.zeros((NB, NTAB), np.float32)
    for b in range(NB):
        cols = np.asarray(edge_index[b, 1], np.int64)
        deg[b, :] = np.bincount(cols, minlength=NTAB).astype(np.float32)
    dinv = np.where(deg > 0, 1.0 / np.sqrt(np.maximum(deg, 1.0)), 0.0).astype(
        np.float32)

    def pmajor(v):
        return np.ascontiguousarray(v.reshape(TILES, 128).T)

    nodes = np.arange(NTAB)
    u_rows = (nodes < N_USERS + 1).astype(np.float32)
    i_rows = ((nodes >= N_USERS + 1) & (nodes < N_NODES)).astype(np.float32)

    bias_rep = np.broadcast_to(
        gcn_b[:, :, None, :], (NB, L, 128, D)).copy().astype(np.float32)
    iota_row = np.broadcast_to(
        np.arange(128, dtype=np.float32), (128, 128)).copy()

    nc = _build_program(plan)
    nc.finalize()

    in_maps = []
    for core in range(N_CORES):
        lo, hi = core * SLICE, (core + 1) * SLICE
        xs = x0[lo:hi]                       # [SLICE, D]
        x0_pm = np.ascontiguousarray(
            xs.reshape(TILES, 128, D).transpose(1, 0, 2).reshape(128, TILES * D))
        m = {
            "x0": x0_pm,
            "dinv": np.stack([pmajor(dinv[b, lo:hi]) for b in range(NB)]),
            "wts": gcn_w,
            "bias": bias_rep,
            "umask": pmajor(u_rows[lo:hi]),
            "imask": pmajor(i_rows[lo:hi]),
            "iota": iota_row,
        }
        for b in range(NB):
            m[f"gidx_{b}"] = core_data[core]["gidx"][b]
            m[f"dcol_{b}"] = core_data[core]["dcol"][b]
        bd = batch_data[core * BS:(core + 1) * BS]  # [BS, NB, 3]
        bi = np.zeros((NB, 3, BS // 128, 128, 1), np.int32)
        for b in range(NB):
            u = _irow(bd[:, b, 0].astype(np.int64)).astype(np.int32)
            p = _irow(N_USERS + 1 + bd[:, b, 1].astype(np.int64)).astype(np.int32)
            n = _irow(N_USERS + 1 + bd[:, b, 2].astype(np.int64)).astype(np.int32)
            for which, v in ((0, u), (1, p), (2, n)):
                bi[b, which] = v.reshape(BS // 128, 128, 1)
        m["bidx"] = bi
        in_maps.append(m)

    trace = _try_install_trace_shim()
    res = run_bass_kernel_spmd(nc, in_maps, list(range(N_CORES)), trace=trace)
    global LAST_EXEC_NS
    LAST_EXEC_NS = res.exec_time_ns
    out = res.results[0]["y"][0]  # [8]
    global LAST_DIAG
    LAST_DIAG = out.copy()
    loss = out[0] / float(B)
    emb = (np.sqrt(out[1]) + np.sqrt(out[2])) / float(N_ITEMS + 1)
    return np.float32(loss + REG_WEIGHT * emb)
